# revision 1
# baseline (speedup 1.0000x reference)
"""Multi-head attention with additive positional bias on 8 Trainium2 cores.

Problem: q,k,v [8, 1024, 512] fp32, pos_bias [1, 8, 1024, 1024] fp32,
8 heads x head_dim 64, out = softmax(q@k^T * scale + bias) @ v.

Sharding: one head per NeuronCore (tensor parallel over heads). The bias
table is per-head, so each core only needs its own 4MB bias slice; q/k/v
slices are 2MB each per core.

Per-core layout trick: compute S^T (scores transposed, j on partitions)
so that
  - matmul 1:  S^T[j,i] = sum_d KT[d,j] * QT[d,i]   (lhsT=KT tile, rhs=QT)
  - softmax:   exp(S^T) * exp(biasT)  elementwise (ACT exp + DVE mul);
               max-subtraction is skipped (scores are ~N(0,1)+-2, safe in fp32)
  - matmul 2:  lhsT=[V|ones] tile [j,65], rhs=P^T -> O^T[dv,i] accumulated
               over j tiles in PSUM; the appended ones-column yields the
               softmax denominators for free in row 64.
All transposes (QT, KT, biasT) and the final divide/untranspose are done
on the host in numpy; the device does only matmuls + exp + mul.

Matmul 1 runs in float32r (full fp32 bit layout, 1 cycle/row at N>=512).
exp output, bias and matmul 2 run in bf16 (errors average out in the PV
reduction; final rel err ~1e-3).
"""

import numpy as np
from contextlib import ExitStack

import concourse.bacc as bacc
import concourse.bass as bass
import concourse.mybir as mybir
import concourse.tile as tile
from concourse.bass_utils import run_bass_kernel_spmd

B = 8          # batch
S = 1024       # sequence length
D = 512        # model dim
H = 8          # heads
HD = 64        # head dim
NT = S // 128  # 128-row j-tiles per sequence
SCALE = HD ** -0.5

_PROGRAM = None


def _emit(ctx, tc, out, qt, kt, vp, eb, repeat=1):
    nc = tc.nc
    f32 = mybir.dt.float32
    f32r = mybir.dt.float32r
    bf16 = mybir.dt.bfloat16

    singles = ctx.enter_context(tc.tile_pool(name="singles", bufs=1))
    qk_pool = ctx.enter_context(tc.tile_pool(name="qk_pool", bufs=2))
    v_pool = ctx.enter_context(tc.tile_pool(name="v_pool", bufs=2))
    e_pool = ctx.enter_context(tc.tile_pool(name="e_pool", bufs=3))
    p_pool = ctx.enter_context(tc.tile_pool(name="p_pool", bufs=3))
    ps_s = ctx.enter_context(tc.tile_pool(name="ps_s", bufs=2, space="PSUM"))
    ps_o = ctx.enter_context(tc.tile_pool(name="ps_o", bufs=2, space="PSUM"))

    # exp(bias^T) stays resident in SBUF: 8 tiles x [128, 1024] bf16 = 16KB/partition
    eb_tiles = []
    for t in range(NT):
        ebt = singles.tile([128, S], bf16, name=f"ebt{t}")
        nc.sync.dma_start(out=ebt, in_=eb[t * 128:(t + 1) * 128, :])
        eb_tiles.append(ebt)

    for b_rep in range(B * repeat):
        b = b_rep % B
        # padded to 128 contraction rows (rows 64..127 are zero): K=64
        # matmuls wedge the device on this runtime, K=128 is also faster.
        qtb = qk_pool.tile([128, S], f32r, tag="qtb")
        nc.sync.dma_start(out=qtb, in_=qt[b])
        ktb = qk_pool.tile([128, S], f32r, tag="ktb")
        nc.sync.dma_start(out=ktb, in_=kt[b])
        vpb = v_pool.tile([128, NT, HD + 1], bf16, tag="vpb")
        nc.sync.dma_start(out=vpb, in_=vp[b])

        po = ps_o.tile([HD + 1, S], f32, tag="po")
        for t in range(NT):
            ps = ps_s.tile([128, S], f32, tag="ps")
            for c in range(2):
                cs = slice(c * 512, (c + 1) * 512)
                # S^T tile: [j=128, i=512] = KT_tile.T @ QT_chunk
                nc.tensor.matmul(
                    ps[:, cs],
                    ktb[:, t * 128:(t + 1) * 128],
                    qtb[:, cs],
                    start=True,
                    stop=True,
                )
            ebf = e_pool.tile([128, S], bf16, tag="ebf")
            nc.scalar.activation(ebf, ps, mybir.ActivationFunctionType.Exp)
            pbf = p_pool.tile([128, S], bf16, tag="pbf")
            nc.vector.tensor_mul(pbf, ebf, eb_tiles[t])
            for c in range(2):
                cs = slice(c * 512, (c + 1) * 512)
                # O^T accum: [dv=65, i=512] += Vpad_tile.T @ P^T_chunk
                nc.tensor.matmul(
                    po[:, cs],
                    vpb[:, t, :],
                    pbf[:, cs],
                    start=(t == 0),
                    stop=(t == NT - 1),
                )
        osb = p_pool.tile([HD + 1, S], f32, tag="osb")
        nc.vector.tensor_copy(osb, po)
        nc.sync.dma_start(out=out[b], in_=osb)


def _build_program(repeat=1):
    nc = bacc.Bacc("TRN2", target_bir_lowering=False, debug=False)
    qt = nc.dram_tensor("qt", [B, 128, S], mybir.dt.float32r, kind="ExternalInput").ap()
    kt = nc.dram_tensor("kt", [B, 128, S], mybir.dt.float32r, kind="ExternalInput").ap()
    vp = nc.dram_tensor(
        "vp", [B, 128, NT, HD + 1], mybir.dt.bfloat16, kind="ExternalInput"
    ).ap()
    eb = nc.dram_tensor("eb", [S, S], mybir.dt.bfloat16, kind="ExternalInput").ap()
    out = nc.dram_tensor("out", [B, HD + 1, S], mybir.dt.float32, kind="ExternalOutput").ap()
    with tile.TileContext(nc) as tc, ExitStack() as ctx:
        _emit(ctx, tc, out, qt, kt, vp, eb, repeat=repeat)
    nc.compile()
    return nc


def get_program(repeat=1):
    global _PROGRAM
    if repeat != 1:
        return _build_program(repeat)
    if _PROGRAM is None:
        _PROGRAM = _build_program()
    return _PROGRAM


def make_in_maps(q, k, v, pos_bias):
    import ml_dtypes

    q4 = q.reshape(B, S, H, HD)
    k4 = k.reshape(B, S, H, HD)
    v4 = v.reshape(B, S, H, HD)
    ones = np.ones((B, S, 1), np.float32)
    in_maps = []
    for h in range(H):
        qt = np.zeros((B, 128, S), np.float32)
        qt[:, :HD, :] = q4[:, :, h, :].transpose(0, 2, 1) * np.float32(SCALE)
        kt = np.zeros((B, 128, S), np.float32)
        kt[:, :HD, :] = k4[:, :, h, :].transpose(0, 2, 1)
        vp = np.concatenate([v4[:, :, h, :], ones], axis=2)  # [B, S, 65]
        vp = np.ascontiguousarray(
            vp.reshape(B, NT, 128, HD + 1).transpose(0, 2, 1, 3)
        ).astype(ml_dtypes.bfloat16)  # [B, 128, NT, 65]
        eb = np.exp(pos_bias[0, h].T).astype(ml_dtypes.bfloat16)  # [S(j), S(i)]
        in_maps.append({"qt": qt, "kt": kt, "vp": vp, "eb": eb})
    return in_maps


def assemble_output(results):
    out = np.empty((B, S, D), np.float32)
    for h in range(H):
        o = results[h]["out"]  # [B, 65, S]
        normed = o[:, :HD, :] / o[:, HD:HD + 1, :]
        out[:, :, h * HD:(h + 1) * HD] = normed.transpose(0, 2, 1)
    return out


def kernel(q, k, v, pos_bias):
    nc = get_program()
    in_maps = make_in_maps(
        np.asarray(q, np.float32),
        np.asarray(k, np.float32),
        np.asarray(v, np.float32),
        np.asarray(pos_bias, np.float32),
    )
    res = run_bass_kernel_spmd(nc, in_maps, list(range(H))).results
    return assemble_output(res)



# revision 2
# speedup vs baseline: 1.0283x; 1.0283x over previous
"""Multi-head attention with additive positional bias on 8 Trainium2 cores.

Problem: q,k,v [8, 1024, 512] fp32, pos_bias [1, 8, 1024, 1024] fp32,
8 heads x head_dim 64, out = softmax(q@k^T * scale + bias) @ v.

Sharding: one head per NeuronCore (tensor parallel over heads).

Per-core pipeline (per batch b, scores transposed: S^T[j,i], j on partitions):
  1. bias inject:  PE matmul  psum[j,i]  = I^T @ biasT_tile   (identity trick)
  2. QK^T:         PE matmul  psum[j,i] += KT_tile^T @ QT     (bf16, K=128)
     -> psum holds s+b in fp32.
  3. drain+exp (the bottleneck, split across two engines):
     - 5/8 j-tiles on ScalarE: true exp, psum fp32 -> sbuf bf16
     - 3/8 j-tiles on VectorE: Schraudolph bit-trick exp:
         bitcast_bf16(int16(A*(s+b) + B)), A=128/ln2, B=127*128+delta
       one tensor_scalar (mult,add) with int16 output aliasing the bf16 tile.
  4. PV:           PE matmul  po[dv,i] += [V|ones]^T @ P^T  accumulated over j
     (65th row of po = softmax denominators via the ones column)
  5. po -> sbuf (VectorE copy), DMA out. Host divides by denominator row and
     untransposes.
"""

import numpy as np
from contextlib import ExitStack

import concourse.bacc as bacc
import concourse.bass as bass
import concourse.mybir as mybir
import concourse.tile as tile
from concourse.bass_utils import run_bass_kernel_spmd

B = 8          # batch
S = 1024       # sequence length
D = 512        # model dim
H = 8          # heads
HD = 64        # head dim
NT = S // 128  # 128-row j-tiles per sequence
SCALE = HD ** -0.5

A_SCH = 128.0 / np.log(2.0)          # Schraudolph scale (bf16: 2^7 mantissa)
B_SCH = 127.0 * 128.0 - 7.0          # exponent bias + minimax-ish delta
SCH_TILES = (1, 4, 7)                # j-tiles drained by the DVE bit-trick exp

_PROGRAM = None


def _emit(ctx, tc, out, qt, kt, vp, bt, ident):
    nc = tc.nc
    f32 = mybir.dt.float32
    bf16 = mybir.dt.bfloat16
    i16 = mybir.dt.int16

    singles = ctx.enter_context(tc.tile_pool(name="singles", bufs=1))
    qk_pool = ctx.enter_context(tc.tile_pool(name="qk_pool", bufs=2))
    v_pool = ctx.enter_context(tc.tile_pool(name="v_pool", bufs=2))
    p_pool = ctx.enter_context(tc.tile_pool(name="p_pool", bufs=9))
    o_pool = ctx.enter_context(tc.tile_pool(name="o_pool", bufs=2))
    ps_s = ctx.enter_context(tc.tile_pool(name="ps_s", bufs=3, space="PSUM"))
    ps_o = ctx.enter_context(tc.tile_pool(name="ps_o", bufs=1, space="PSUM"))

    # resident: bias^T tiles (8 x [128, 1024] bf16 = 16KB/partition) + identity
    idt = singles.tile([128, 128], bf16, name="idt")
    nc.sync.dma_start(out=idt, in_=ident)
    bt_tiles = []
    for t in range(NT):
        btt = singles.tile([128, S], bf16, name=f"btt{t}")
        nc.sync.dma_start(out=btt, in_=bt[t * 128:(t + 1) * 128, :])
        bt_tiles.append(btt)

    for b in range(B):
        qtb = qk_pool.tile([128, S], bf16, tag="qtb")
        nc.sync.dma_start(out=qtb, in_=qt[b])
        ktb = qk_pool.tile([128, S], bf16, tag="ktb")
        nc.sync.dma_start(out=ktb, in_=kt[b])
        vpb = v_pool.tile([128, NT, HD + 1], bf16, tag="vpb")
        nc.sync.dma_start(out=vpb, in_=vp[b])

        ptiles = []
        for t in range(NT):
            ps = ps_s.tile([128, S], f32, tag="ps")
            for c in range(2):
                cs = slice(c * 512, (c + 1) * 512)
                # bias inject: psum[j, i] = biasT[j, i] (identity stationary)
                nc.tensor.matmul(
                    ps[:, cs], idt, bt_tiles[t][:, cs], start=True, stop=False
                )
                # scores: psum[j, i] += KT_tile.T @ QT_chunk
                nc.tensor.matmul(
                    ps[:, cs],
                    ktb[:, t * 128:(t + 1) * 128],
                    qtb[:, cs],
                    start=False,
                    stop=True,
                )
            pt = p_pool.tile([128, S], bf16, tag="pt")
            if t in SCH_TILES:
                nc.vector.tensor_scalar(
                    pt.bitcast(i16),
                    ps,
                    A_SCH,
                    B_SCH,
                    mybir.AluOpType.mult,
                    mybir.AluOpType.add,
                )
            else:
                nc.scalar.activation(pt, ps, mybir.ActivationFunctionType.Exp)
            ptiles.append(pt)

        po = ps_o.tile([HD + 1, S], f32, tag="po")
        for t in range(NT):
            for c in range(2):
                cs = slice(c * 512, (c + 1) * 512)
                # O^T accum: [dv=65, i=512] += Vpad_tile.T @ P^T_chunk
                nc.tensor.matmul(
                    po[:, cs],
                    vpb[:, t, :],
                    ptiles[t][:, cs],
                    start=(t == 0),
                    stop=(t == NT - 1),
                )
        osb = o_pool.tile([HD + 1, S], f32, tag="osb")
        nc.vector.tensor_copy(osb, po)
        nc.sync.dma_start(out=out[b], in_=osb)


def _build_program():
    nc = bacc.Bacc("TRN2", target_bir_lowering=False, debug=False)
    bf16 = mybir.dt.bfloat16
    qt = nc.dram_tensor("qt", [B, 128, S], bf16, kind="ExternalInput").ap()
    kt = nc.dram_tensor("kt", [B, 128, S], bf16, kind="ExternalInput").ap()
    vp = nc.dram_tensor("vp", [B, 128, NT, HD + 1], bf16, kind="ExternalInput").ap()
    bt = nc.dram_tensor("bt", [S, S], bf16, kind="ExternalInput").ap()
    ident = nc.dram_tensor("ident", [128, 128], bf16, kind="ExternalInput").ap()
    out = nc.dram_tensor("out", [B, HD + 1, S], mybir.dt.float32, kind="ExternalOutput").ap()
    with tile.TileContext(nc) as tc, ExitStack() as ctx:
        _emit(ctx, tc, out, qt, kt, vp, bt, ident)
    nc.compile()
    return nc


def get_program():
    global _PROGRAM
    if _PROGRAM is None:
        _PROGRAM = _build_program()
    return _PROGRAM


def make_in_maps(q, k, v, pos_bias):
    import ml_dtypes

    nbf16 = ml_dtypes.bfloat16
    q4 = q.reshape(B, S, H, HD)
    k4 = k.reshape(B, S, H, HD)
    v4 = v.reshape(B, S, H, HD)
    ones = np.ones((B, S, 1), np.float32)
    ident = np.eye(128, dtype=nbf16)
    in_maps = []
    for h in range(H):
        qt = np.zeros((B, 128, S), nbf16)
        qt[:, :HD, :] = (q4[:, :, h, :].transpose(0, 2, 1) * np.float32(SCALE)).astype(
            nbf16
        )
        kt = np.zeros((B, 128, S), nbf16)
        kt[:, :HD, :] = k4[:, :, h, :].transpose(0, 2, 1).astype(nbf16)
        vp = np.concatenate([v4[:, :, h, :], ones], axis=2)  # [B, S, 65]
        vp = np.ascontiguousarray(
            vp.reshape(B, NT, 128, HD + 1).transpose(0, 2, 1, 3)
        ).astype(nbf16)  # [B, 128, NT, 65]
        bt = np.ascontiguousarray(pos_bias[0, h].T).astype(nbf16)  # [S(j), S(i)]
        in_maps.append({"qt": qt, "kt": kt, "vp": vp, "bt": bt, "ident": ident})
    return in_maps


def assemble_output(results):
    out = np.empty((B, S, D), np.float32)
    for h in range(H):
        o = results[h]["out"]  # [B, 65, S]
        normed = o[:, :HD, :] / o[:, HD:HD + 1, :]
        out[:, :, h * HD:(h + 1) * HD] = normed.transpose(0, 2, 1)
    return out


def kernel(q, k, v, pos_bias):
    nc = get_program()
    in_maps = make_in_maps(
        np.asarray(q, np.float32),
        np.asarray(k, np.float32),
        np.asarray(v, np.float32),
        np.asarray(pos_bias, np.float32),
    )
    res = run_bass_kernel_spmd(nc, in_maps, list(range(H))).results
    return assemble_output(res)


# revision 5
# speedup vs baseline: 1.0952x; 1.0650x over previous
"""Multi-head attention with additive positional bias on 8 Trainium2 cores.

Problem: q,k,v [8, 1024, 512] fp32, pos_bias [1, 8, 1024, 1024] fp32,
8 heads x head_dim 64, out = softmax(q@k^T * scale + bias) @ v.

Sharding: one head per NeuronCore (tensor parallel over heads).

Per-core pipeline (scores transposed: S^T[j,i], j on partitions; q is
pre-scaled by A*SCALE on the host so psum holds A*s where A=128/ln2):
  1. QK^T on PE, bf16, K=64 row-tiled: j-tile pairs co-execute on the two
     64-row halves of the PE array (qt/kt rows 64:128 duplicate 0:64).
  2. drain+exp (the bottleneck), split across engines per j-tile:
     - ACT tiles: true exp via ScalarE (scale=1/A), then *exp(bias) on
       VectorE or GpSimd (bf16 2x tensor_tensor).
     - DVE tiles: Schraudolph bit-trick exp with bias folded in:
       bitcast_bf16(int16(A*s + (A*bias + B))) -- one tensor_add against a
       resident fp32 table, int16 output aliasing the bf16 P tile.
  3. PV on PE: po[dv,i] += [V|ones]^T @ P^T accumulated over j-tiles
     (65th row = softmax denominators). mm2 for batch b-1 is interleaved
     between batch b's QK matmuls so the PE never stalls on drains.
  4. po -> sbuf (ScalarE copy), DMA out; host divides and untransposes.
"""

import numpy as np
from contextlib import ExitStack

import concourse.bacc as bacc
import concourse.bass as bass
import concourse.mybir as mybir
import concourse.tile as tile
from concourse.bass_utils import run_bass_kernel_spmd

B = 8          # batch
S = 1024       # sequence length
D = 512        # model dim
H = 8          # heads
HD = 64        # head dim
NT = S // 128  # 128-row j-tiles per sequence
SCALE = HD ** -0.5

A_SCH = 128.0 / np.log(2.0)          # Schraudolph scale (bf16: 2^7 mantissa)
B_SCH = 127.0 * 128.0 - 7.0          # exponent bias + minimax-ish delta
SCH_TILES = (1, 3, 5)                # j-tiles drained by the DVE bit-trick exp
GPS_MUL_TILES = (0, 4)               # exp-tiles whose bias-mul runs on GpSimd
ROW_TILED = True                     # K=64 PE row-tiling for QK^T

_PROGRAM = None


def _emit(ctx, tc, out, qt, kt, vp, bb, eb):
    nc = tc.nc
    f32 = mybir.dt.float32
    bf16 = mybir.dt.bfloat16
    i16 = mybir.dt.int16

    singles = ctx.enter_context(tc.tile_pool(name="singles", bufs=1))
    qk_pool = ctx.enter_context(tc.tile_pool(name="qk_pool", bufs=2))
    v_pool = ctx.enter_context(tc.tile_pool(name="v_pool", bufs=2))
    e_pool = ctx.enter_context(tc.tile_pool(name="e_pool", bufs=6))
    p_pool = ctx.enter_context(tc.tile_pool(name="p_pool", bufs=12))
    o_pool = ctx.enter_context(tc.tile_pool(name="o_pool", bufs=2))
    ps_s = ctx.enter_context(tc.tile_pool(name="ps_s", bufs=3, space="PSUM"))
    ps_o = ctx.enter_context(tc.tile_pool(name="ps_o", bufs=1, space="PSUM"))

    # resident tables: Bb fp32 (A*bias^T + B) 32KB/part, eb bf16 16KB/part
    bb_tiles, eb_tiles = [], []
    for t in range(NT):
        bbt = singles.tile([128, S], f32, name=f"bbt{t}")
        nc.sync.dma_start(out=bbt, in_=bb[t * 128:(t + 1) * 128, :])
        bb_tiles.append(bbt)
        ebt = singles.tile([128, S], bf16, name=f"ebt{t}")
        nc.sync.dma_start(out=ebt, in_=eb[t * 128:(t + 1) * 128, :])
        eb_tiles.append(ebt)

    prev = None  # (ptiles, vpb) of previous batch, for interleaved mm2

    def mm1(qtb, ktb, t):
        """QK^T for j-tile t -> psum tile (A*s, fp32)."""
        ps = ps_s.tile([128, S], f32, tag="ps")
        if ROW_TILED:
            r = slice(64, 128) if (t % 2) else slice(0, 64)
        else:
            r = slice(0, 128)
        for c in range(2):
            cs = slice(c * 512, (c + 1) * 512)
            nc.tensor.matmul(
                ps[:, cs],
                ktb[r, t * 128:(t + 1) * 128],
                qtb[r, cs],
                start=True,
                stop=True,
            )
        return ps

    def drain(ps, t):
        """psum (A*s) -> P tile (bf16 ~ exp(s+b))."""
        pt = p_pool.tile([128, S], bf16, tag="pt")
        if t in SCH_TILES:
            nc.vector.tensor_add(pt.bitcast(i16), ps, bb_tiles[t])
        else:
            et = e_pool.tile([128, S], bf16, tag="et")
            nc.scalar.activation(
                et, ps, mybir.ActivationFunctionType.Exp, scale=float(1.0 / A_SCH)
            )
            eng = nc.gpsimd if t in GPS_MUL_TILES else nc.vector
            eng.tensor_mul(pt, et, eb_tiles[t])
        return pt

    def mm2(po, ptiles, vpb, t):
        for c in range(2):
            cs = slice(c * 512, (c + 1) * 512)
            nc.tensor.matmul(
                po[:, cs],
                vpb[:, t, :],
                ptiles[t][:, cs],
                start=(t == 0),
                stop=(t == NT - 1),
            )

    def finish(po_prev):
        osb = o_pool.tile([HD + 1, S], f32, tag="osb")
        nc.scalar.activation(osb, po_prev[0], mybir.ActivationFunctionType.Copy)
        nc.sync.dma_start(out=out[po_prev[1]], in_=osb)

    for b in range(B):
        qtb = qk_pool.tile([128, S], bf16, tag="qtb")
        nc.sync.dma_start(out=qtb, in_=qt[b])
        ktb = qk_pool.tile([128, S], bf16, tag="ktb")
        nc.sync.dma_start(out=ktb, in_=kt[b])
        vpb = v_pool.tile([128, NT, HD + 1], bf16, tag="vpb")
        nc.sync.dma_start(out=vpb, in_=vp[b])

        po = None
        if prev is not None:
            po = ps_o.tile([HD + 1, S], f32, tag="po")
        ptiles = []
        for p in range(NT // 2):
            t0, t1 = 2 * p, 2 * p + 1
            # pair emitted back-to-back on PE so the K=64 halves co-execute
            ps0 = mm1(qtb, ktb, t0)
            ps1 = mm1(qtb, ktb, t1)
            if prev is not None:
                mm2(po, prev[0], prev[1], t0)
                mm2(po, prev[0], prev[1], t1)
            ptiles.append(drain(ps0, t0))
            ptiles.append(drain(ps1, t1))
        if prev is not None:
            finish((po, b - 1))
        prev = (ptiles, vpb)

    po = ps_o.tile([HD + 1, S], f32, tag="po")
    for t in range(NT):
        mm2(po, prev[0], prev[1], t)
    finish((po, B - 1))


def _build_program():
    nc = bacc.Bacc("TRN2", target_bir_lowering=False, debug=False)
    bf16 = mybir.dt.bfloat16
    qt = nc.dram_tensor("qt", [B, 128, S], bf16, kind="ExternalInput").ap()
    kt = nc.dram_tensor("kt", [B, 128, S], bf16, kind="ExternalInput").ap()
    vp = nc.dram_tensor("vp", [B, 128, NT, HD + 1], bf16, kind="ExternalInput").ap()
    bb = nc.dram_tensor("bb", [S, S], mybir.dt.float32, kind="ExternalInput").ap()
    eb = nc.dram_tensor("eb", [S, S], bf16, kind="ExternalInput").ap()
    out = nc.dram_tensor("out", [B, HD + 1, S], mybir.dt.float32, kind="ExternalOutput").ap()
    with tile.TileContext(nc) as tc, ExitStack() as ctx:
        _emit(ctx, tc, out, qt, kt, vp, bb, eb)
    nc.compile()
    return nc


def get_program():
    global _PROGRAM
    if _PROGRAM is None:
        _PROGRAM = _build_program()
    return _PROGRAM


def make_in_maps(q, k, v, pos_bias):
    import ml_dtypes

    nbf16 = ml_dtypes.bfloat16
    q4 = q.reshape(B, S, H, HD)
    k4 = k.reshape(B, S, H, HD)
    v4 = v.reshape(B, S, H, HD)
    ones = np.ones((B, S, 1), np.float32)
    qscale = np.float32(SCALE * A_SCH)
    in_maps = []
    for h in range(H):
        qt = np.empty((B, 128, S), nbf16)
        qt[:, :HD, :] = (q4[:, :, h, :].transpose(0, 2, 1) * qscale).astype(nbf16)
        qt[:, HD:, :] = qt[:, :HD, :]
        kt = np.empty((B, 128, S), nbf16)
        kt[:, :HD, :] = k4[:, :, h, :].transpose(0, 2, 1).astype(nbf16)
        kt[:, HD:, :] = kt[:, :HD, :]
        vp = np.concatenate([v4[:, :, h, :], ones], axis=2)  # [B, S, 65]
        vp = np.ascontiguousarray(
            vp.reshape(B, NT, 128, HD + 1).transpose(0, 2, 1, 3)
        ).astype(nbf16)  # [B, 128, NT, 65]
        btT = np.ascontiguousarray(pos_bias[0, h].T).astype(np.float32)  # [j, i]
        bb = (A_SCH * btT.astype(nbf16).astype(np.float32) + B_SCH).astype(np.float32)
        eb = np.exp(btT.astype(nbf16).astype(np.float32)).astype(nbf16)
        in_maps.append({"qt": qt, "kt": kt, "vp": vp, "bb": bb, "eb": eb})
    return in_maps


def assemble_output(results):
    out = np.empty((B, S, D), np.float32)
    for h in range(H):
        o = results[h]["out"]  # [B, 65, S]
        normed = o[:, :HD, :] / o[:, HD:HD + 1, :]
        out[:, :, h * HD:(h + 1) * HD] = normed.transpose(0, 2, 1)
    return out


def kernel(q, k, v, pos_bias):
    nc = get_program()
    in_maps = make_in_maps(
        np.asarray(q, np.float32),
        np.asarray(k, np.float32),
        np.asarray(v, np.float32),
        np.asarray(pos_bias, np.float32),
    )
    res = run_bass_kernel_spmd(nc, in_maps, list(range(H))).results
    return assemble_output(res)
